# revision 12
# baseline (speedup 1.0000x reference)
"""Multi-head attention kernel for Trainium2, SPMD over 8 NeuronCores.

Problem: B=4, S=4096, E=256, H=4 heads (dh=64), f32.
  q = target @ Wq ; k = source @ Wk ; v = source @ Wv   (per-head slices)
  out = softmax(q k^T / sqrt(dh)) v  -> concat heads -> @ Wo

Sharding: core c handles batch b = c//2 and heads (2*(c%2), 2*(c%2)+1).
Each core computes, for its two heads, the transposed unnormalized
attention output u^T = V^T @ exp(K Q^T / 8) of shape [64, 4096] plus the
softmax denominators (via a ones-column appended to V inside the PV
matmul). Host applies the cheap parts: QKV projections (tiny GEMMs),
the final normalization, and the output projection + cross-head sum.

Device data layout (per core):
  qt: [128, 4096] f32   rows 0-63 head-A Q^T, rows 64-127 head-B Q^T
  kt: [128, 4096] f32   same for K^T
  v:  [128, 2, 32, 65]  v[p,h,j,0:64] = V[j*128+p, :] for head h; [...,64]=1
  out:[2, 65, 4096]     out[h,0:64,q] = u_h^T, out[h,64,q] = sum_k exp
"""

import numpy as np

S = 4096
E = 256
H = 4
DH = 64
NJ = S // 128   # 32 key chunks of 128
QB = 512        # q block width (per head, per step)
NQB = S // QB   # 8
NCORES = 8

_CACHE = {}


def _build_nc():
    import concourse.mybir as mybir
    import concourse.tile as tile
    from concourse import bacc

    f32 = mybir.dt.float32
    bf16 = mybir.dt.bfloat16
    EXP = mybir.ActivationFunctionType.Exp

    nc = bacc.Bacc("TRN2", target_bir_lowering=False, debug=False)

    qt_d = nc.dram_tensor("qt", [128, S], bf16, kind="ExternalInput").ap()
    kt_d = nc.dram_tensor("kt", [128, S], bf16, kind="ExternalInput").ap()
    v_d = nc.dram_tensor("v", [128, 2, NJ, 65], bf16, kind="ExternalInput").ap()
    out_d = nc.dram_tensor("out", [2, 65, S], f32, kind="ExternalOutput").ap()

    # Everything runs in the PE's 64x128 row-tiled mode (contraction 64,
    # two concurrent tiles at partition offsets 0 and 64) — no 128-mode
    # matmuls anywhere, so the PE never drains for a tiling-mode switch
    # and the HAM clock ramps to 2.4 GHz.
    #   QK: head A on tile (0,0), head B on tile (64,0), same j chunk.
    #   PV: each head's 128-row key chunk is split into two 64-row halves
    #       (tiles (0,0)/(64,0)); each half carries its own ones column,
    #       and top+bot partial outputs are summed on the DVE at evac.
    with tile.TileContext(nc) as tc:
        with (
            tc.tile_pool(name="const", bufs=1) as const,
            tc.tile_pool(name="expp", bufs=3) as expp,
            tc.tile_pool(name="evp", bufs=2) as evp,
            tc.tile_pool(name="qkp", bufs=2, space="PSUM") as qkp,
            tc.tile_pool(name="pvp", bufs=1, space="PSUM") as pvp,
        ):
            # First QK needs only qt block 0 and kt chunks 0-1; land those
            # with small DMAs so the pipeline starts ~5us earlier.
            qt = const.tile([128, S], bf16)
            nc.sync.dma_start(qt[:, 0:QB], qt_d[:, 0:QB])
            kt = const.tile([128, S], bf16)
            nc.sync.dma_start(kt[:, 0:512], kt_d[:, 0:512])
            nc.sync.dma_start(qt[:, QB:], qt_d[:, QB:])
            nc.sync.dma_start(kt[:, 512:], kt_d[:, 512:])
            vsb = const.tile([128, 2, NJ, 65], bf16)
            nc.sync.dma_start(vsb[:], v_d)

            # Software-pipelined emission: the PE instruction stream must
            # issue QK(s+2) BEFORE PV(s) so the ACTIVATE pipeline never
            # starves — QK(s+2) is what unblocks ACT(s+2), while the PVs
            # have ~2us of slack. (QK(s+2) reuses the PSUM slab ACT(s)
            # reads, so it becomes ready exactly when ACT(s) completes.)
            steps = [(qb, j) for qb in range(NQB) for j in range(NJ)]
            pv = {}
            exs = {}
            for s in range(len(steps) + 2):
                if s < len(steps):
                    qb, j = steps[s]
                    qsl = slice(qb * QB, (qb + 1) * QB)
                    jsl = slice(j * 128, (j + 1) * 128)
                    qk = qkp.tile([128, 2 * QB], f32)
                    # head A scores -> cols [0:QB], head B -> cols [QB:2QB]
                    for h in range(2):
                        psl = slice(h * 64, (h + 1) * 64)
                        nc.tensor.matmul(
                            qk[:, h * QB:(h + 1) * QB],
                            kt[psl, jsl],
                            qt[psl, qsl],
                            start=True,
                            stop=True,
                        )
                    ex = expp.tile([128, 2 * QB], bf16)
                    nc.scalar.activation(ex[:], qk[:], EXP, scale=0.125)
                    exs[s] = ex
                if s >= 2:
                    qb, j = steps[s - 2]
                    ex = exs.pop(s - 2)
                    if j == 0:
                        for h in range(2):
                            for part in range(2):
                                pv[h, part] = pvp.tile(
                                    [65, QB], f32,
                                    tag=f"pv_{h}_{part}", name=f"pv_{h}_{part}",
                                )
                    for h in range(2):
                        for part in range(2):
                            psl = slice(part * 64, (part + 1) * 64)
                            nc.tensor.matmul(
                                pv[h, part][:, :],
                                vsb[psl, h, j, :],
                                ex[psl, h * QB:(h + 1) * QB],
                                start=(j == 0),
                                stop=(j == NJ - 1),
                            )
                    if j == NJ - 1:
                        qsl = slice(qb * QB, (qb + 1) * QB)
                        for h in range(2):
                            ev = evp.tile([65, QB], f32)
                            nc.vector.tensor_copy(ev[:], pv[h, 0][:, :])
                            nc.vector.tensor_add(ev[:], ev[:], pv[h, 1][:, :])
                            nc.sync.dma_start(out_d[h, :, qsl], ev[:])

    nc.compile()
    return nc


def _get_nc():
    if "nc" not in _CACHE:
        _CACHE["nc"] = _build_nc()
    return _CACHE["nc"]


def kernel(target, source, Wq, Wk, Wv, Wo):
    from concourse.bass_utils import run_bass_kernel_spmd

    target = np.asarray(target, dtype=np.float32)
    source = np.asarray(source, dtype=np.float32)
    Wq = np.asarray(Wq, dtype=np.float32)
    Wk = np.asarray(Wk, dtype=np.float32)
    Wv = np.asarray(Wv, dtype=np.float32)
    Wo = np.asarray(Wo, dtype=np.float32)
    B = target.shape[0]

    import ml_dtypes

    bf16 = ml_dtypes.bfloat16
    in_maps = []
    for c in range(NCORES):
        b = c // 2
        h0 = (c % 2) * 2
        cols = slice(h0 * DH, (h0 + 2) * DH)  # 128 cols = 2 heads
        q = target[b] @ Wq[:, cols]           # [S, 128]
        k = source[b] @ Wk[:, cols]           # [S, 128]
        v = source[b] @ Wv[:, cols]           # [S, 128]
        vv = np.ones((128, 2, NJ, 65), bf16)
        # v[p, h, j, 0:64] = V[j*128+p, h*64:(h+1)*64]
        vr = v.reshape(NJ, 128, 2, DH)        # [j, p, h, d]
        vv[:, :, :, :DH] = vr.transpose(1, 2, 0, 3).astype(bf16)
        in_maps.append(
            {
                "qt": np.ascontiguousarray(q.T.astype(bf16)),
                "kt": np.ascontiguousarray(k.T.astype(bf16)),
                "v": vv,
            }
        )

    nc = _get_nc()
    res = run_bass_kernel_spmd(nc, in_maps, core_ids=list(range(NCORES)))

    out = np.zeros((B, S, E), np.float32)
    for c in range(NCORES):
        b = c // 2
        h0 = (c % 2) * 2
        u = res.results[c]["out"]  # [2, 65, S]
        for hh in range(2):
            att_t = u[hh, :DH, :] / u[hh, DH:DH + 1, :]   # [64, S]
            out[b] += att_t.T @ Wo[(h0 + hh) * DH:(h0 + hh + 1) * DH, :]
    return out


# revision 13
# speedup vs baseline: 1.0134x; 1.0134x over previous
"""Multi-head attention kernel for Trainium2, SPMD over 8 NeuronCores.

Problem: B=4, S=4096, E=256, H=4 heads (dh=64), f32.
  q = target @ Wq ; k = source @ Wk ; v = source @ Wv   (per-head slices)
  out = softmax(q k^T / sqrt(dh)) v  -> concat heads -> @ Wo

Sharding: core c handles batch b = c//2 and heads (2*(c%2), 2*(c%2)+1).
Each core computes, for its two heads, the transposed unnormalized
attention output u^T = V^T @ exp(K Q^T / 8) of shape [64, 4096] plus the
softmax denominators (via a ones-column appended to V inside the PV
matmul). Host applies the cheap parts: QKV projections (tiny GEMMs),
the final normalization, and the output projection + cross-head sum.

Device data layout (per core):
  qt: [128, 4096] f32   rows 0-63 head-A Q^T, rows 64-127 head-B Q^T
  kt: [128, 4096] f32   same for K^T
  v:  [128, 2, 32, 65]  v[p,h,j,0:64] = V[j*128+p, :] for head h; [...,64]=1
  out:[2, 65, 4096]     out[h,0:64,q] = u_h^T, out[h,64,q] = sum_k exp
"""

import numpy as np

S = 4096
E = 256
H = 4
DH = 64
NJ = S // 128   # 32 key chunks of 128
QB = 512        # q block width (per head, per step)
NQB = S // QB   # 8
NCORES = 8

_CACHE = {}


def _build_nc():
    import concourse.mybir as mybir
    import concourse.tile as tile
    from concourse import bacc

    f32 = mybir.dt.float32
    bf16 = mybir.dt.bfloat16
    EXP = mybir.ActivationFunctionType.Exp

    nc = bacc.Bacc("TRN2", target_bir_lowering=False, debug=False)

    qt_d = nc.dram_tensor("qt", [128, S], bf16, kind="ExternalInput").ap()
    kt_d = nc.dram_tensor("kt", [128, S], bf16, kind="ExternalInput").ap()
    v_d = nc.dram_tensor("v", [128, 2, NJ, 65], bf16, kind="ExternalInput").ap()
    out_d = nc.dram_tensor("out", [2, 65, S], f32, kind="ExternalOutput").ap()

    # Everything runs in the PE's 64x128 row-tiled mode (contraction 64,
    # two concurrent tiles at partition offsets 0 and 64) — no 128-mode
    # matmuls anywhere, so the PE never drains for a tiling-mode switch
    # and the HAM clock ramps to 2.4 GHz.
    #   QK: head A on tile (0,0), head B on tile (64,0), same j chunk.
    #   PV: each head's 128-row key chunk is split into two 64-row halves
    #       (tiles (0,0)/(64,0)); each half carries its own ones column,
    #       and top+bot partial outputs are summed on the DVE at evac.
    with tile.TileContext(nc) as tc:
        with (
            tc.tile_pool(name="const", bufs=1) as const,
            tc.tile_pool(name="expp", bufs=4) as expp,
            tc.tile_pool(name="evp", bufs=2) as evp,
            tc.tile_pool(name="qkp", bufs=2, space="PSUM") as qkp,
            tc.tile_pool(name="pvp", bufs=1, space="PSUM") as pvp,
        ):
            # First QK needs only qt block 0 and kt chunks 0-1; land those
            # with small DMAs so the pipeline starts ~5us earlier.
            qt = const.tile([128, S], bf16)
            nc.sync.dma_start(qt[:, 0:QB], qt_d[:, 0:QB])
            kt = const.tile([128, S], bf16)
            nc.sync.dma_start(kt[:, 0:512], kt_d[:, 0:512])
            nc.sync.dma_start(qt[:, QB:], qt_d[:, QB:])
            nc.sync.dma_start(kt[:, 512:], kt_d[:, 512:])
            vsb = const.tile([128, 2, NJ, 65], bf16)
            nc.sync.dma_start(vsb[:], v_d)

            # Software-pipelined emission: the PE instruction stream must
            # issue QK(s+2) BEFORE PV(s) so the ACTIVATE pipeline never
            # starves — QK(s+2) is what unblocks ACT(s+2), while the PVs
            # have ~2us of slack. (QK(s+2) reuses the PSUM slab ACT(s)
            # reads, so it becomes ready exactly when ACT(s) completes.)
            steps = [(qb, j) for qb in range(NQB) for j in range(NJ)]
            pv = {}
            exs = {}
            for s in range(len(steps) + 2):
                if s < len(steps):
                    qb, j = steps[s]
                    qsl = slice(qb * QB, (qb + 1) * QB)
                    jsl = slice(j * 128, (j + 1) * 128)
                    qk = qkp.tile([128, 2 * QB], f32)
                    # head A scores -> cols [0:QB], head B -> cols [QB:2QB]
                    for h in range(2):
                        psl = slice(h * 64, (h + 1) * 64)
                        nc.tensor.matmul(
                            qk[:, h * QB:(h + 1) * QB],
                            kt[psl, jsl],
                            qt[psl, qsl],
                            start=True,
                            stop=True,
                        )
                    ex = expp.tile([128, 2 * QB], bf16)
                    nc.scalar.activation(ex[:], qk[:], EXP, scale=0.125)
                    exs[s] = ex
                if s >= 2:
                    qb, j = steps[s - 2]
                    ex = exs.pop(s - 2)
                    if j == 0:
                        for h in range(2):
                            for part in range(2):
                                pv[h, part] = pvp.tile(
                                    [65, QB], f32,
                                    tag=f"pv_{h}_{part}", name=f"pv_{h}_{part}",
                                )
                    for h in range(2):
                        for part in range(2):
                            psl = slice(part * 64, (part + 1) * 64)
                            nc.tensor.matmul(
                                pv[h, part][:, :],
                                vsb[psl, h, j, :],
                                ex[psl, h * QB:(h + 1) * QB],
                                start=(j == 0),
                                stop=(j == NJ - 1),
                            )
                    if j == NJ - 1:
                        qsl = slice(qb * QB, (qb + 1) * QB)
                        for h in range(2):
                            ev = evp.tile([65, QB], f32)
                            nc.vector.tensor_copy(ev[:], pv[h, 0][:, :])
                            nc.vector.tensor_add(ev[:], ev[:], pv[h, 1][:, :])
                            nc.sync.dma_start(out_d[h, :, qsl], ev[:])

    nc.compile()
    return nc


def _get_nc():
    if "nc" not in _CACHE:
        _CACHE["nc"] = _build_nc()
    return _CACHE["nc"]


def kernel(target, source, Wq, Wk, Wv, Wo):
    from concourse.bass_utils import run_bass_kernel_spmd

    target = np.asarray(target, dtype=np.float32)
    source = np.asarray(source, dtype=np.float32)
    Wq = np.asarray(Wq, dtype=np.float32)
    Wk = np.asarray(Wk, dtype=np.float32)
    Wv = np.asarray(Wv, dtype=np.float32)
    Wo = np.asarray(Wo, dtype=np.float32)
    B = target.shape[0]

    import ml_dtypes

    bf16 = ml_dtypes.bfloat16
    in_maps = []
    for c in range(NCORES):
        b = c // 2
        h0 = (c % 2) * 2
        cols = slice(h0 * DH, (h0 + 2) * DH)  # 128 cols = 2 heads
        q = target[b] @ Wq[:, cols]           # [S, 128]
        k = source[b] @ Wk[:, cols]           # [S, 128]
        v = source[b] @ Wv[:, cols]           # [S, 128]
        vv = np.ones((128, 2, NJ, 65), bf16)
        # v[p, h, j, 0:64] = V[j*128+p, h*64:(h+1)*64]
        vr = v.reshape(NJ, 128, 2, DH)        # [j, p, h, d]
        vv[:, :, :, :DH] = vr.transpose(1, 2, 0, 3).astype(bf16)
        in_maps.append(
            {
                "qt": np.ascontiguousarray(q.T.astype(bf16)),
                "kt": np.ascontiguousarray(k.T.astype(bf16)),
                "v": vv,
            }
        )

    nc = _get_nc()
    res = run_bass_kernel_spmd(nc, in_maps, core_ids=list(range(NCORES)))

    out = np.zeros((B, S, E), np.float32)
    for c in range(NCORES):
        b = c // 2
        h0 = (c % 2) * 2
        u = res.results[c]["out"]  # [2, 65, S]
        for hh in range(2):
            att_t = u[hh, :DH, :] / u[hh, DH:DH + 1, :]   # [64, S]
            out[b] += att_t.T @ Wo[(h0 + hh) * DH:(h0 + hh + 1) * DH, :]
    return out


# revision 14
# speedup vs baseline: 1.0159x; 1.0025x over previous
"""Multi-head attention kernel for Trainium2, SPMD over 8 NeuronCores.

Problem: B=4, S=4096, E=256, H=4 heads (dh=64), f32.
  q = target @ Wq ; k = source @ Wk ; v = source @ Wv   (per-head slices)
  out = softmax(q k^T / sqrt(dh)) v  -> concat heads -> @ Wo

Sharding: core c handles batch b = c//2 and heads (2*(c%2), 2*(c%2)+1).
Each core computes, for its two heads, the transposed unnormalized
attention output u^T = V^T @ exp(K Q^T / 8) of shape [64, 4096] plus the
softmax denominators (via a ones-column appended to V inside the PV
matmul). Host applies the cheap parts: QKV projections (tiny GEMMs),
the final normalization, and the output projection + cross-head sum.

Device data layout (per core):
  qt: [128, 4096] f32   rows 0-63 head-A Q^T, rows 64-127 head-B Q^T
  kt: [128, 4096] f32   same for K^T
  v:  [128, 2, 32, 65]  v[p,h,j,0:64] = V[j*128+p, :] for head h; [...,64]=1
  out:[2, 65, 4096]     out[h,0:64,q] = u_h^T, out[h,64,q] = sum_k exp
"""

import numpy as np

S = 4096
E = 256
H = 4
DH = 64
NJ = S // 128   # 32 key chunks of 128
QB = 512        # q block width (per head, per step)
NQB = S // QB   # 8
NCORES = 8

_CACHE = {}


def _build_nc():
    import concourse.mybir as mybir
    import concourse.tile as tile
    from concourse import bacc

    f32 = mybir.dt.float32
    bf16 = mybir.dt.bfloat16
    EXP = mybir.ActivationFunctionType.Exp

    nc = bacc.Bacc("TRN2", target_bir_lowering=False, debug=False)

    qt_d = nc.dram_tensor("qt", [128, S], bf16, kind="ExternalInput").ap()
    kt_d = nc.dram_tensor("kt", [128, S], bf16, kind="ExternalInput").ap()
    v_d = nc.dram_tensor("v", [128, 2, NJ, 65], bf16, kind="ExternalInput").ap()
    out_d = nc.dram_tensor("out", [2, 65, S], f32, kind="ExternalOutput").ap()

    # Everything runs in the PE's 64x128 row-tiled mode (contraction 64,
    # two concurrent tiles at partition offsets 0 and 64) — no 128-mode
    # matmuls anywhere, so the PE never drains for a tiling-mode switch
    # and the HAM clock ramps to 2.4 GHz.
    #   QK: head A on tile (0,0), head B on tile (64,0), same j chunk.
    #   PV: each head's 128-row key chunk is split into two 64-row halves
    #       (tiles (0,0)/(64,0)); each half carries its own ones column,
    #       and top+bot partial outputs are summed on the DVE at evac.
    with tile.TileContext(nc) as tc:
        with (
            tc.tile_pool(name="const", bufs=1) as const,
            tc.tile_pool(name="expp", bufs=5) as expp,
            tc.tile_pool(name="evp", bufs=2) as evp,
            tc.tile_pool(name="qkp", bufs=2, space="PSUM") as qkp,
            tc.tile_pool(name="pvp", bufs=1, space="PSUM") as pvp,
        ):
            # First QK needs only qt block 0 and kt chunks 0-1; land those
            # with small DMAs so the pipeline starts ~5us earlier.
            qt = const.tile([128, S], bf16)
            nc.sync.dma_start(qt[:, 0:QB], qt_d[:, 0:QB])
            kt = const.tile([128, S], bf16)
            nc.sync.dma_start(kt[:, 0:512], kt_d[:, 0:512])
            nc.sync.dma_start(qt[:, QB:], qt_d[:, QB:])
            nc.sync.dma_start(kt[:, 512:], kt_d[:, 512:])
            vsb = const.tile([128, 2, NJ, 65], bf16)
            nc.sync.dma_start(vsb[:], v_d)

            # Software-pipelined emission: the PE instruction stream must
            # issue QK(s+2) BEFORE PV(s) so the ACTIVATE pipeline never
            # starves — QK(s+2) is what unblocks ACT(s+2), while the PVs
            # have ~2us of slack. (QK(s+2) reuses the PSUM slab ACT(s)
            # reads, so it becomes ready exactly when ACT(s) completes.)
            steps = [(qb, j) for qb in range(NQB) for j in range(NJ)]
            pv = {}
            exs = {}
            for s in range(len(steps) + 2):
                if s < len(steps):
                    qb, j = steps[s]
                    qsl = slice(qb * QB, (qb + 1) * QB)
                    jsl = slice(j * 128, (j + 1) * 128)
                    qk = qkp.tile([128, 2 * QB], f32)
                    # head A scores -> cols [0:QB], head B -> cols [QB:2QB]
                    for h in range(2):
                        psl = slice(h * 64, (h + 1) * 64)
                        nc.tensor.matmul(
                            qk[:, h * QB:(h + 1) * QB],
                            kt[psl, jsl],
                            qt[psl, qsl],
                            start=True,
                            stop=True,
                        )
                    ex = expp.tile([128, 2 * QB], bf16)
                    nc.scalar.activation(ex[:], qk[:], EXP, scale=0.125)
                    exs[s] = ex
                if s >= 2:
                    qb, j = steps[s - 2]
                    ex = exs.pop(s - 2)
                    if j == 0:
                        for h in range(2):
                            for part in range(2):
                                pv[h, part] = pvp.tile(
                                    [65, QB], f32,
                                    tag=f"pv_{h}_{part}", name=f"pv_{h}_{part}",
                                )
                    for h in range(2):
                        for part in range(2):
                            psl = slice(part * 64, (part + 1) * 64)
                            nc.tensor.matmul(
                                pv[h, part][:, :],
                                vsb[psl, h, j, :],
                                ex[psl, h * QB:(h + 1) * QB],
                                start=(j == 0),
                                stop=(j == NJ - 1),
                            )
                    if j == NJ - 1:
                        qsl = slice(qb * QB, (qb + 1) * QB)
                        for h in range(2):
                            ev = evp.tile([65, QB], f32)
                            nc.vector.tensor_copy(ev[:], pv[h, 0][:, :])
                            nc.vector.tensor_add(ev[:], ev[:], pv[h, 1][:, :])
                            nc.sync.dma_start(out_d[h, :, qsl], ev[:])

    nc.compile()
    return nc


def _get_nc():
    if "nc" not in _CACHE:
        _CACHE["nc"] = _build_nc()
    return _CACHE["nc"]


def kernel(target, source, Wq, Wk, Wv, Wo):
    from concourse.bass_utils import run_bass_kernel_spmd

    target = np.asarray(target, dtype=np.float32)
    source = np.asarray(source, dtype=np.float32)
    Wq = np.asarray(Wq, dtype=np.float32)
    Wk = np.asarray(Wk, dtype=np.float32)
    Wv = np.asarray(Wv, dtype=np.float32)
    Wo = np.asarray(Wo, dtype=np.float32)
    B = target.shape[0]

    import ml_dtypes

    bf16 = ml_dtypes.bfloat16
    in_maps = []
    for c in range(NCORES):
        b = c // 2
        h0 = (c % 2) * 2
        cols = slice(h0 * DH, (h0 + 2) * DH)  # 128 cols = 2 heads
        q = target[b] @ Wq[:, cols]           # [S, 128]
        k = source[b] @ Wk[:, cols]           # [S, 128]
        v = source[b] @ Wv[:, cols]           # [S, 128]
        vv = np.ones((128, 2, NJ, 65), bf16)
        # v[p, h, j, 0:64] = V[j*128+p, h*64:(h+1)*64]
        vr = v.reshape(NJ, 128, 2, DH)        # [j, p, h, d]
        vv[:, :, :, :DH] = vr.transpose(1, 2, 0, 3).astype(bf16)
        in_maps.append(
            {
                "qt": np.ascontiguousarray(q.T.astype(bf16)),
                "kt": np.ascontiguousarray(k.T.astype(bf16)),
                "v": vv,
            }
        )

    nc = _get_nc()
    res = run_bass_kernel_spmd(nc, in_maps, core_ids=list(range(NCORES)))

    out = np.zeros((B, S, E), np.float32)
    for c in range(NCORES):
        b = c // 2
        h0 = (c % 2) * 2
        u = res.results[c]["out"]  # [2, 65, S]
        for hh in range(2):
            att_t = u[hh, :DH, :] / u[hh, DH:DH + 1, :]   # [64, S]
            out[b] += att_t.T @ Wo[(h0 + hh) * DH:(h0 + hh + 1) * DH, :]
    return out
